# revision 5
# baseline (speedup 1.0000x reference)
"""DDPM scheduler kernel for Trainium2 (Bass/Tile), 8-core data parallel.

Computes out = exp(clog[clip(round(t), 0, 1000)]) for t in [0, 1000],
where clog is the cumulative-log-alpha table of the classical DDPM
beta schedule (beta0=1e-4, beta1T/T=0.02, T=1000).

Instead of a 1001-entry table gather (slow on TRN2), we evaluate a
degree-4 polynomial fit of clog(n) (max |err| 3.2e-8 in log domain,
far below the fp32 table's own ~1.1e-5 noise floor vs the exact curve):

    n  = rint(t)                       (DVE, magic-number round-to-nearest-even)
    u  = n / 1024
    P4(u) = SE * [(u+H1)^2 + O1] * [(u+H2)^2 + O2]
    out = exp(P4)

The two quadratic factors are the real-root pair and complex-root pair
of the quartic fit; each is one ACT Square (free fused scale+bias) plus
one scalar add. Per 2.1M-element core: DVE 3 passes, ACT 3 passes,
GPSIMD 1 pass -- every engine under the ~47us HBM roofline.
"""

import numpy as np

import concourse.bacc as bacc
import concourse.mybir as mybir
from concourse.bass_utils import run_bass_kernel_spmd
from concourse.tile import TileContext

N_CORES = 8
TOTAL = 16777216
PER_CORE = TOTAL // N_CORES  # 2097152
P = 128

# fp32 constants (derived offline from the exact fp64 table; see module docstring)
MAGIC = 12582912.0  # 1.5 * 2^23: (t + MAGIC) - MAGIC == rint(t) for 0 <= t < 2^22
SCALE = float(np.float32(2.0**-10))
H1 = float(np.float32(0.0044141756))
O1 = float(np.float32(-1.9481873e-05))
H2 = float(np.float32(47.5497))
O2 = float(np.float32(11728.624))
SE = float(np.float32(-0.0007465615))


def build_nc(per_core: int = PER_CORE, chunk_f: int = 4096):
    assert per_core % (P * chunk_f) == 0
    n_chunks = per_core // (P * chunk_f)

    # Bacc (not raw Bass): its finalize() runs generate_event_semaphores(),
    # which splits multi-sem waits into InstEventSemaphore chains -- TRN2
    # allows at most 1 sync-wait per compute instruction.
    nc = bacc.Bacc()
    t_in = nc.dram_tensor("t", [per_core], mybir.dt.float32, kind="ExternalInput")
    y_out = nc.dram_tensor("y", [per_core], mybir.dt.float32, kind="ExternalOutput")
    t_r = t_in.rearrange("(c p f) -> c p f", p=P, f=chunk_f)
    y_r = y_out.rearrange("(c p f) -> c p f", p=P, f=chunk_f)

    AF = mybir.ActivationFunctionType
    OP = mybir.AluOpType
    f32 = mybir.dt.float32

    with TileContext(nc) as tc:
        with (
            tc.tile_pool(name="const", bufs=1) as const_pool,
            tc.tile_pool(name="io", bufs=3) as io_pool,
            tc.tile_pool(name="wk", bufs=2) as wk_pool,
        ):
            b1 = const_pool.tile([P, 1], f32, tag="b1")
            nc.gpsimd.memset(b1[:], H1)
            b2 = const_pool.tile([P, 1], f32, tag="b2")
            nc.gpsimd.memset(b2[:], H2)
            for ci in range(n_chunks):
                tt = io_pool.tile([P, chunk_f], f32, tag="t")
                nc.sync.dma_start(tt[:], t_r[ci])
                # n = rint(t), exact for round-half-to-even (matches jnp.round)
                nc.vector.tensor_scalar(
                    tt[:], tt[:], MAGIC, MAGIC, OP.add, OP.subtract
                )
                # factor 1: (u + H1)^2 + O1   (u = n/1024 via ACT's fused scale)
                y1 = wk_pool.tile([P, chunk_f], f32, tag="y1")
                nc.scalar.activation(y1[:], tt[:], AF.Square, bias=b1[:], scale=SCALE)
                nc.vector.tensor_scalar(y1[:], y1[:], O1, None, OP.add)
                # factor 2: (u + H2)^2 + O2
                y2 = io_pool.tile([P, chunk_f], f32, tag="y2")
                nc.scalar.activation(y2[:], tt[:], AF.Square, bias=b2[:], scale=SCALE)
                nc.gpsimd.tensor_scalar(y2[:], y2[:], O2, None, OP.add)
                # W = factor1 * factor2 ; out = exp(SE * W)
                nc.vector.tensor_tensor(y1[:], y1[:], y2[:], OP.mult)
                nc.scalar.activation(y2[:], y1[:], AF.Exp, bias=0.0, scale=SE)
                nc.sync.dma_start(y_r[ci], y2[:])
    # Bacc.finalize() runs compile() (reg alloc, event-sem legalization);
    # run_bass_via_pjrt serializes nc as-is and needs this done.
    nc.finalize()
    return nc


_nc_cache = None


def kernel(t: np.ndarray) -> np.ndarray:
    global _nc_cache
    assert t.shape == (TOTAL,) and t.dtype == np.float32
    if _nc_cache is None:
        _nc_cache = build_nc()
    nc = _nc_cache
    shards = np.ascontiguousarray(t.reshape(N_CORES, PER_CORE))
    in_maps = [{"t": shards[i]} for i in range(N_CORES)]
    res = run_bass_kernel_spmd(nc, in_maps, core_ids=list(range(N_CORES)))
    return np.concatenate([r["y"] for r in res.results])


# revision 6
# speedup vs baseline: 3.9352x; 3.9352x over previous
"""DDPM scheduler kernel for Trainium2 (Bass/Tile), 8-core data parallel.

Computes out = exp(clog[clip(round(t), 0, 1000)]) for t in [0, 1000],
where clog is the cumulative-log-alpha table of the classical DDPM
beta schedule (beta0=1e-4, beta1T/T=0.02, T=1000).

Instead of a 1001-entry table gather (slow on TRN2), we evaluate a
degree-4 polynomial fit of clog(n) (max |err| 3.2e-8 in log domain,
far below the fp32 table's own ~1.1e-5 noise floor vs the exact curve):

    n  = rint(t)                       (DVE, magic-number round-to-nearest-even)
    u  = n / 1024
    P4(u) = SE * [(u+H1)^2 + O1] * [(u+H2)^2 + O2]
    out = exp(P4)

The two quadratic factors are the real-root pair and complex-root pair
of the quartic fit; each is one ACT Square (free fused scale+bias) plus
one scalar add. Per 2.1M-element core: DVE 3 passes, ACT 3 passes,
GPSIMD 1 pass -- every engine under the ~47us HBM roofline.
"""

import numpy as np

import concourse.bacc as bacc
import concourse.mybir as mybir
from concourse.bass_utils import run_bass_kernel_spmd
from concourse.tile import TileContext

N_CORES = 8
TOTAL = 16777216
PER_CORE = TOTAL // N_CORES  # 2097152
P = 128

# fp32 constants (derived offline from the exact fp64 table; see module docstring)
MAGIC = 12582912.0  # 1.5 * 2^23: (t + MAGIC) - MAGIC == rint(t) for 0 <= t < 2^22
SCALE = float(np.float32(2.0**-10))
H1 = float(np.float32(0.0044141756))
O1 = float(np.float32(-1.9481873e-05))
H2 = float(np.float32(47.5497))
O2 = float(np.float32(11728.624))
SE = float(np.float32(-0.0007465615))


def build_nc(per_core: int = PER_CORE, chunk_f: int = 4096):
    assert per_core % (P * chunk_f) == 0
    n_chunks = per_core // (P * chunk_f)

    # Bacc (not raw Bass): its finalize() runs generate_event_semaphores(),
    # which splits multi-sem waits into InstEventSemaphore chains -- TRN2
    # allows at most 1 sync-wait per compute instruction.
    nc = bacc.Bacc()
    t_in = nc.dram_tensor("t", [per_core], mybir.dt.float32, kind="ExternalInput")
    y_out = nc.dram_tensor("y", [per_core], mybir.dt.float32, kind="ExternalOutput")
    t_r = t_in.rearrange("(c p f) -> c p f", p=P, f=chunk_f)
    y_r = y_out.rearrange("(c p f) -> c p f", p=P, f=chunk_f)

    AF = mybir.ActivationFunctionType
    OP = mybir.AluOpType
    f32 = mybir.dt.float32

    with TileContext(nc) as tc:
        with (
            tc.tile_pool(name="const", bufs=1) as const_pool,
            tc.tile_pool(name="io", bufs=3) as io_pool,
            tc.tile_pool(name="wk", bufs=2) as wk_pool,
        ):
            b1 = const_pool.tile([P, 1], f32, tag="b1")
            nc.gpsimd.memset(b1[:], H1)
            b2 = const_pool.tile([P, 1], f32, tag="b2")
            nc.gpsimd.memset(b2[:], H2)
            for ci in range(n_chunks):
                tt = io_pool.tile([P, chunk_f], f32, tag="t")
                nc.sync.dma_start(tt[:], t_r[ci])
                # n = rint(t), exact for round-half-to-even (matches jnp.round)
                nc.vector.tensor_scalar(
                    tt[:], tt[:], MAGIC, MAGIC, OP.add, OP.subtract
                )
                # factor 1: (u + H1)^2 + O1   (u = n/1024 via ACT's fused scale)
                y1 = wk_pool.tile([P, chunk_f], f32, tag="y1")
                nc.scalar.activation(y1[:], tt[:], AF.Square, bias=b1[:], scale=SCALE)
                nc.vector.tensor_scalar(y1[:], y1[:], O1, None, OP.add)
                # factor 2: (u + H2)^2 + O2
                y2 = io_pool.tile([P, chunk_f], f32, tag="y2")
                nc.scalar.activation(y2[:], tt[:], AF.Square, bias=b2[:], scale=SCALE)
                # NOT gpsimd: its tensor_scalar runs ~17x slower than DVE and
                # its SBUF-port sharing stretches concurrent DVE ops to match
                # (measured 2.3us -> 60us).
                nc.vector.tensor_scalar(y2[:], y2[:], O2, None, OP.add)
                # W = factor1 * factor2 ; out = exp(SE * W)
                nc.vector.tensor_tensor(y1[:], y1[:], y2[:], OP.mult)
                nc.scalar.activation(y2[:], y1[:], AF.Exp, bias=0.0, scale=SE)
                nc.sync.dma_start(y_r[ci], y2[:])
    # Bacc.finalize() runs compile() (reg alloc, event-sem legalization);
    # run_bass_via_pjrt serializes nc as-is and needs this done.
    nc.finalize()
    return nc


_nc_cache = None


def kernel(t: np.ndarray) -> np.ndarray:
    global _nc_cache
    assert t.shape == (TOTAL,) and t.dtype == np.float32
    if _nc_cache is None:
        _nc_cache = build_nc()
    nc = _nc_cache
    shards = np.ascontiguousarray(t.reshape(N_CORES, PER_CORE))
    in_maps = [{"t": shards[i]} for i in range(N_CORES)]
    res = run_bass_kernel_spmd(nc, in_maps, core_ids=list(range(N_CORES)))
    return np.concatenate([r["y"] for r in res.results])
